# revision 12
# baseline (speedup 1.0000x reference)
"""Trainium2 Bass kernel for MultiHeadAttention (B=4, S=1024, D=1024, H=16).

Sharding: 8 cores = (batch b in 0..3) x (head-group g in 0..1, 8 heads each).
Each core computes, for its (b, g):
  - qhT/khT = (Wq_g/8) @ q[b]^T, Wk_g @ k[b]^T   (head-transposed projections)
  - vh      = v[b] @ Wv_g^T                       (natural layout, + ones column)
  - per head: logitsT[sk,sq] = khT^T-style matmul + adjoinT (mask folded in),
              expT = exp(logitsT)  (written unnormalized to HBM),
              ctx~T/Z via single matmul against ones-augmented vh,
              ctxT = ctx~T * (1/Z) + wv_b
  - out_partial[sq,e] = ctxT_g^T @ dense_w_g^T    (row-parallel dense)
Host: pre-transposes inputs/weights, then normalizes+transposes attention
weights and sum-reduces the two dense partials per batch.
"""

import sys

if "/opt/trn_rl_repo" not in sys.path:
    sys.path.insert(0, "/opt/trn_rl_repo")

import numpy as np

import concourse.bass as bass  # noqa: F401  (registers types)
import concourse.tile as tile
from concourse import bacc, mybir
from concourse.bass_utils import run_bass_kernel_spmd

P = 128
S = 1024
D = 1024
H = 16
HC = 8        # heads per core
DEPTH = 64
W = 512       # local (per-core) head width = HC * DEPTH
F32 = mybir.dt.float32

# Matmul compute dtype: float32 (exact) or float32r (4x faster, reduced
# precision multiplies). Chosen empirically against the fp32 reference.
MM_DT = mybir.dt.float32r


def _mm(ap, mm_dt):
    return ap.bitcast(mm_dt) if mm_dt != F32 else ap


def build_program(mm_dt=MM_DT, es_dt=None):
    if es_dt is None:
        es_dt = mm_dt
    nc = bacc.Bacc("TRN2", target_bir_lowering=False, debug=False, num_devices=8)

    qT = nc.dram_tensor("qT", [D, S], mm_dt, kind="ExternalInput").ap()
    kT = nc.dram_tensor("kT", [D, S], mm_dt, kind="ExternalInput").ap()
    vT = nc.dram_tensor("vT", [D, S], mm_dt, kind="ExternalInput").ap()
    adjT_d = nc.dram_tensor("adjT", [S, S], mm_dt, kind="ExternalInput").ap()
    wqT = nc.dram_tensor("wqT", [D, W], mm_dt, kind="ExternalInput").ap()
    wkT = nc.dram_tensor("wkT", [D, W], mm_dt, kind="ExternalInput").ap()
    wvT = nc.dram_tensor("wvT", [D, W], mm_dt, kind="ExternalInput").ap()
    dwT_d = nc.dram_tensor("dwT", [W, D], mm_dt, kind="ExternalInput").ap()
    qb_d = nc.dram_tensor("qb", [P, 4], F32, kind="ExternalInput").ap()
    kb_d = nc.dram_tensor("kb", [P, 4], F32, kind="ExternalInput").ap()
    vb_d = nc.dram_tensor("vb", [P, 4], F32, kind="ExternalInput").ap()

    attn_un = nc.dram_tensor("attn_un", [HC, S, S], es_dt, kind="ExternalOutput").ap()
    outp = nc.dram_tensor("outp", [S, D], F32, kind="ExternalOutput").ap()

    ADD = mybir.AluOpType.add
    MULT = mybir.AluOpType.mult
    EXP = mybir.ActivationFunctionType.Exp

    with tile.TileContext(nc) as tc:
        with tc.tile_pool(name="const", bufs=1) as const, \
             tc.tile_pool(name="persist", bufs=1) as persist:
            ones = const.tile([P, DEPTH], F32, tag="ones")
            nc.vector.memset(ones[:], 1.0)
            from concourse.masks import make_identity
            ident_f32 = const.tile([P, P], F32, tag="identf")
            make_identity(nc, ident_f32)
            if mm_dt != F32:
                ident = const.tile([P, P], mm_dt, tag="ident")
                nc.vector.tensor_copy(out=ident[:], in_=ident_f32[:])
            else:
                ident = ident_f32
            qb_sb = const.tile([P, 4], F32, tag="qb")
            kb_sb = const.tile([P, 4], F32, tag="kb")
            vb_sb = const.tile([P, 4], F32, tag="vb")
            nc.gpsimd.dma_start(out=qb_sb[:], in_=qb_d[:])
            nc.gpsimd.dma_start(out=kb_sb[:], in_=kb_d[:])
            nc.gpsimd.dma_start(out=vb_sb[:], in_=vb_d[:])

            qhT = [persist.tile([P, S], mm_dt, tag=f"qhT{m}", name=f"qhT{m}") for m in range(4)]
            khT = [persist.tile([P, S], mm_dt, tag=f"khT{m}", name=f"khT{m}") for m in range(4)]
            vh = [persist.tile([P, HC, DEPTH + 1], es_dt, tag=f"vh{m}", name=f"vh{m}")
                  for m in range(8)]
            ctxT = [persist.tile([P, S], mm_dt, tag=f"ctxT{m}", name=f"ctxT{m}") for m in range(4)]
            adjT = [persist.tile([P, S], mm_dt, tag=f"adjT{m}", name=f"adjT{m}") for m in range(8)]
            dwT = [persist.tile([P, S], mm_dt, tag=f"dwT{m}", name=f"dwT{m}") for m in range(4)]

            for m in range(8):
                nc.gpsimd.dma_start(out=adjT[m][:], in_=adjT_d[m * P:(m + 1) * P, :])
            for m in range(4):
                nc.gpsimd.dma_start(out=dwT[m][:], in_=dwT_d[m * P:(m + 1) * P, :])
            for m in range(8):
                nc.vector.tensor_copy(out=vh[m][:, :, DEPTH:DEPTH + 1],
                                      in_=ones[:, 0:HC])

            # ---- Phase A: projections ----
            with tc.tile_pool(name="acts", bufs=3) as actp, \
                 tc.tile_pool(name="wts", bufs=8) as wtp, \
                 tc.tile_pool(name="psA", bufs=1, space="PSUM") as psA:

                def proj_qkT(src, w_src, bias_sb, outT):
                    ps = {}
                    for m in range(4):
                        for n in range(2):
                            ps[(m, n)] = psA.tile([P, 512], F32, tag=f"pj{m * 2 + n}", name=f"pj{m * 2 + n}")
                    for kd in range(8):
                        w_t = wtp.tile([P, W], mm_dt, tag="w")
                        nc.gpsimd.dma_start(out=w_t[:],
                                          in_=w_src[kd * P:(kd + 1) * P, :])
                        a_t = actp.tile([P, S], mm_dt, tag="a")
                        nc.gpsimd.dma_start(out=a_t[:],
                                          in_=src[kd * P:(kd + 1) * P, :])
                        for m in range(4):
                            for n in range(2):
                                nc.tensor.matmul(
                                    ps[(m, n)][:],
                                    w_t[:, m * P:(m + 1) * P],
                                    a_t[:, n * 512:(n + 1) * 512],
                                    start=(kd == 0), stop=(kd == 7))
                    for m in range(4):
                        for n in range(2):
                            nc.vector.tensor_scalar(
                                out=outT[m][:, n * 512:(n + 1) * 512],
                                in0=ps[(m, n)][:],
                                scalar1=bias_sb[:, m:m + 1],
                                scalar2=None, op0=ADD)

                def proj_v():
                    ps = [psA.tile([P, W], F32, tag=f"pj{i}", name=f"pj{i}") for i in range(8)]
                    for kd in range(8):
                        w_t = wtp.tile([P, W], mm_dt, tag="w")
                        nc.gpsimd.dma_start(out=w_t[:],
                                          in_=wvT[kd * P:(kd + 1) * P, :])
                        a_t = actp.tile([P, S], mm_dt, tag="a")
                        nc.gpsimd.dma_start(out=a_t[:],
                                          in_=vT[kd * P:(kd + 1) * P, :])
                        for m in range(8):
                            nc.tensor.matmul(
                                ps[m][:],
                                a_t[:, m * P:(m + 1) * P],
                                w_t[:],
                                start=(kd == 0), stop=(kd == 7))
                    for m in range(8):
                        nc.vector.tensor_copy(
                            out=vh[m][:, :, 0:DEPTH],
                            in_=ps[m][:].rearrange("p (h d) -> p h d", h=HC))

                proj_qkT(qT, wqT, qb_sb, qhT)
                proj_qkT(kT, wkT, kb_sb, khT)
                proj_v()

            # ---- Phase B: attention, head pairs ----
            # Heads 2j (partitions 0:64) and 2j+1 (partitions 64:128) of
            # chunk j are processed together: their K=64 logits matmuls use
            # disjoint PE row-groups (tile_position auto-derived from the
            # base partition) and run concurrently. The adjoin add runs
            # in-place in PSUM on the DVE; exp reads PSUM directly, so no
            # PE instruction waits on DVE/ACT within a pass.
            with tc.tile_pool(name="esb", bufs=5) as esp, \
                 tc.tile_pool(name="zsb", bufs=2) as zsp, \
                 tc.tile_pool(name="plp", bufs=2, space="PSUM") as plp, \
                 tc.tile_pool(name="pcp", bufs=1, space="PSUM") as pcp:
                for j in range(HC // 2):
                    hs = (2 * j, 2 * j + 1)
                    q_h = {h: qhT[j][(h % 2) * DEPTH:(h % 2 + 1) * DEPTH, :]
                           for h in hs}
                    k_h = {h: khT[j][(h % 2) * DEPTH:(h % 2 + 1) * DEPTH, :]
                           for h in hs}
                    pcs = {h: pcp.tile([DEPTH + 1, S], F32, tag=f"pc{h % 2}",
                                       name=f"pc{h % 2}") for h in hs}
                    es_tiles = {}
                    for skc in range(2):      # sk chunks of 4
                        sks = range(4 * skc, 4 * skc + 4)
                        # pass 1: logits (PE, paired row-groups) -> +adjoin
                        # (DVE, in-place in PSUM) -> exp (ACT, PSUM->SBUF)
                        es_pair = {}
                        for sk in sks:
                            pls = {}
                            for n in range(2):
                                for h in hs:
                                    pl = plp.tile([P, 512], F32,
                                                  tag=f"pl{h % 2}",
                                                  name=f"pl{h % 2}")
                                    pls[(h, n)] = pl
                                    nc.tensor.matmul(
                                        pl[:],
                                        k_h[h][:, sk * P:(sk + 1) * P],
                                        q_h[h][:, n * 512:(n + 1) * 512],
                                        start=True, stop=True)
                            for h in hs:
                                if sk % 2 == 0:
                                    es_pair[h] = esp.tile([P, 2, S], es_dt,
                                                          tag="es", name="es")
                                es_t = es_pair[h]
                                es_tiles[(h, sk)] = es_t[:, sk % 2, :]
                                for n in range(2):
                                    pl = pls[(h, n)]
                                    nc.vector.tensor_tensor(
                                        out=pl[:], in0=pl[:],
                                        in1=adjT[sk][:, n * 512:(n + 1) * 512],
                                        op=ADD)
                                    nc.scalar.activation(
                                        es_t[:, sk % 2, n * 512:(n + 1) * 512],
                                        pl[:], EXP)
                                if sk % 2 == 1:
                                    a0 = (sk - 1) * P
                                    dst = attn_un[h, a0:a0 + 2 * P, :]
                                    dst = dst.rearrange("(s p) e -> p s e",
                                                        p=P)
                                    nc.sync.dma_start(out=dst, in_=es_t[:])
                        # pass 2: ctx~T accumulation
                        for h in hs:
                            for sk in sks:
                                for n in range(2):
                                    nc.tensor.matmul(
                                        pcs[h][:, n * 512:(n + 1) * 512],
                                        vh[sk][:, h, :],
                                        es_tiles[(h, sk)][:, n * 512:(n + 1) * 512],
                                        start=(sk == 0), stop=(sk == 7))
                    # Z -> 1/Z -> partition broadcast -> normalize ctx~T
                    for h in hs:
                        poff = (h % 2) * DEPTH
                        pc = pcs[h]
                        zz = zsp.tile([1, S], F32, tag="zz")
                        nc.vector.tensor_copy(out=zz[0:1, :],
                                              in_=pc[DEPTH:DEPTH + 1, :])
                        rz = zsp.tile([1, S], F32, tag="rz")
                        nc.vector.reciprocal_approx_fast(out=rz[0:1, :],
                                                         in_=zz[0:1, :])
                        rzb = zsp.tile([DEPTH, S], F32, tag="rzb")
                        nc.gpsimd.partition_broadcast(rzb[:], rz[0:1, :],
                                                      channels=DEPTH)
                        ct = ctxT[j]
                        nc.vector.tensor_tensor(out=ct[poff:poff + DEPTH, :],
                                                in0=pc[0:DEPTH, :],
                                                in1=rzb[:], op=MULT)
                        nc.vector.tensor_scalar(
                            out=ct[poff:poff + DEPTH, :],
                            in0=ct[poff:poff + DEPTH, :],
                            scalar1=vb_sb[poff:poff + DEPTH, j:j + 1],
                            scalar2=None, op0=ADD)

            # ---- Phase C: dense (row-parallel partial) ----
            with tc.tile_pool(name="osb", bufs=3) as osp, \
                 tc.tile_pool(name="psC", bufs=4, space="PSUM") as psC:
                for m in range(8):
                    os_t = osp.tile([P, S], F32, tag="os")
                    for n in range(2):
                        pd = psC.tile([P, 512], F32, tag="pd")
                        for kc in range(4):
                            nc.tensor.matmul(
                                pd[:],
                                ctxT[kc][:, m * P:(m + 1) * P],
                                dwT[kc][:, n * 512:(n + 1) * 512],
                                start=(kc == 0), stop=(kc == 3))
                        nc.vector.tensor_copy(
                            out=os_t[:, n * 512:(n + 1) * 512], in_=pd[:])
                    nc.sync.dma_start(out=outp[m * P:(m + 1) * P, :],
                                      in_=os_t[:])

    nc.compile()
    return nc


_PROGRAM_CACHE = {}


def get_program(mm_dt=MM_DT, es_dt=None):
    key = (str(mm_dt), str(es_dt))
    if key not in _PROGRAM_CACHE:
        _PROGRAM_CACHE[key] = build_program(mm_dt, es_dt)
    return _PROGRAM_CACHE[key]


def make_in_maps(v, k, q, mask, adjoin_matrix,
                 wq_w, wq_b, wk_w, wk_b, wv_w, wv_b, dense_w, dense_b):
    c = np.ascontiguousarray
    f32 = np.float32
    in_maps = []
    per_batch = {}
    for b in range(4):
        per_batch[b] = {
            "qT": c(np.asarray(q[b], f32).T),
            "kT": c(np.asarray(k[b], f32).T),
            "vT": c(np.asarray(v[b], f32).T),
            "adjT": c(np.asarray(adjoin_matrix[b, 0], f32).T)
            + np.float32(-1e9) * np.asarray(mask[b, 0, 0], f32)[:, None],
        }
    for cid in range(8):
        b, g = cid // 2, cid % 2
        gs = slice(g * W, (g + 1) * W)
        m = dict(per_batch[b])
        m["wqT"] = c(np.asarray(wq_w, f32)[gs].T) * f32(0.125)
        m["wkT"] = c(np.asarray(wk_w, f32)[gs].T)
        m["wvT"] = c(np.asarray(wv_w, f32)[gs].T)
        m["dwT"] = c(np.asarray(dense_w, f32)[:, gs].T)
        m["qb"] = c((np.asarray(wq_b, f32)[gs] * f32(0.125)).reshape(4, P).T)
        m["kb"] = c(np.asarray(wk_b, f32)[gs].reshape(4, P).T)
        m["vb"] = c(np.asarray(wv_b, f32)[gs].reshape(4, P).T)
        in_maps.append(m)
    return in_maps


def assemble_outputs(results, dense_b):
    out = np.empty((4, S, D), np.float32)
    attn = np.empty((4, H, S, S), np.float32)
    for cid in range(8):
        b, g = cid // 2, cid % 2
        au = results[cid]["attn_un"]          # [HC, sk, sq]
        z = au.sum(axis=1)                    # [HC, sq]
        attn[b, g * HC:(g + 1) * HC] = (au / z[:, None, :]).transpose(0, 2, 1)
    db = np.asarray(dense_b, np.float32)
    for b in range(4):
        out[b] = results[2 * b]["outp"] + results[2 * b + 1]["outp"] + db
    return out, attn


def run_cores(inputs, mm_dt=MM_DT, es_dt=None, trace=False, **run_kwargs):
    nc = get_program(mm_dt, es_dt)
    in_maps = make_in_maps(**inputs)
    res = run_bass_kernel_spmd(nc, in_maps, core_ids=list(range(8)),
                               trace=trace, **run_kwargs)
    return res


def kernel(**inputs):
    res = run_cores(inputs)
    return assemble_outputs(res.results, inputs["dense_b"])


# revision 14
# speedup vs baseline: 1.1350x; 1.1350x over previous
"""Trainium2 Bass kernel for MultiHeadAttention (B=4, S=1024, D=1024, H=16).

Sharding: 8 cores = (batch b in 0..3) x (head-group g in 0..1, 8 heads each).
Each core computes, for its (b, g):
  - qhT/khT = (Wq_g/8) @ q[b]^T, Wk_g @ k[b]^T   (head-transposed projections)
  - vh      = v[b] @ Wv_g^T                       (natural layout, + ones column)
  - per head: logitsT[sk,sq] = khT^T-style matmul + adjoinT (mask folded in),
              expT = exp(logitsT)  (written unnormalized to HBM),
              ctx~T/Z via single matmul against ones-augmented vh,
              ctxT = ctx~T * (1/Z) + wv_b
  - out_partial[sq,e] = ctxT_g^T @ dense_w_g^T    (row-parallel dense)
Host: pre-transposes inputs/weights, then normalizes+transposes attention
weights and sum-reduces the two dense partials per batch.
"""

import sys

if "/opt/trn_rl_repo" not in sys.path:
    sys.path.insert(0, "/opt/trn_rl_repo")

import numpy as np

import concourse.bass as bass  # noqa: F401  (registers types)
import concourse.tile as tile
from concourse import bacc, mybir
from concourse.bass_utils import run_bass_kernel_spmd

P = 128
S = 1024
D = 1024
H = 16
HC = 8        # heads per core
DEPTH = 64
W = 512       # local (per-core) head width = HC * DEPTH
F32 = mybir.dt.float32

# Matmul compute dtype: float32 (exact) or float32r (4x faster, reduced
# precision multiplies). Chosen empirically against the fp32 reference.
MM_DT = mybir.dt.float32r


def _mm(ap, mm_dt):
    return ap.bitcast(mm_dt) if mm_dt != F32 else ap


def build_program(mm_dt=MM_DT, es_dt=None):
    if es_dt is None:
        es_dt = mm_dt
    nc = bacc.Bacc("TRN2", target_bir_lowering=False, debug=False, num_devices=8)

    qT = nc.dram_tensor("qT", [D, S], mm_dt, kind="ExternalInput").ap()
    kT = nc.dram_tensor("kT", [D, S], mm_dt, kind="ExternalInput").ap()
    vT = nc.dram_tensor("vT", [D, S], mm_dt, kind="ExternalInput").ap()
    adjT_d = nc.dram_tensor("adjT", [S, S], mm_dt, kind="ExternalInput").ap()
    wqT = nc.dram_tensor("wqT", [D, W], mm_dt, kind="ExternalInput").ap()
    wkT = nc.dram_tensor("wkT", [D, W], mm_dt, kind="ExternalInput").ap()
    wvT = nc.dram_tensor("wvT", [D, W], mm_dt, kind="ExternalInput").ap()
    dwT_d = nc.dram_tensor("dwT", [W, D], mm_dt, kind="ExternalInput").ap()
    qb_d = nc.dram_tensor("qb", [P, 4], F32, kind="ExternalInput").ap()
    kb_d = nc.dram_tensor("kb", [P, 4], F32, kind="ExternalInput").ap()
    vb_d = nc.dram_tensor("vb", [P, 4], F32, kind="ExternalInput").ap()

    attn_un = nc.dram_tensor("attn_un", [HC, S, S], es_dt, kind="ExternalOutput").ap()
    outp = nc.dram_tensor("outp", [S, D], F32, kind="ExternalOutput").ap()

    ADD = mybir.AluOpType.add
    MULT = mybir.AluOpType.mult
    EXP = mybir.ActivationFunctionType.Exp

    with tile.TileContext(nc) as tc:
        with tc.tile_pool(name="const", bufs=1) as const, \
             tc.tile_pool(name="persist", bufs=1) as persist:
            ones = const.tile([P, DEPTH], F32, tag="ones")
            nc.vector.memset(ones[:], 1.0)
            from concourse.masks import make_identity
            ident_f32 = const.tile([P, P], F32, tag="identf")
            make_identity(nc, ident_f32)
            if mm_dt != F32:
                ident = const.tile([P, P], mm_dt, tag="ident")
                nc.vector.tensor_copy(out=ident[:], in_=ident_f32[:])
            else:
                ident = ident_f32
            qb_sb = const.tile([P, 4], F32, tag="qb")
            kb_sb = const.tile([P, 4], F32, tag="kb")
            vb_sb = const.tile([P, 4], F32, tag="vb")
            nc.gpsimd.dma_start(out=qb_sb[:], in_=qb_d[:])
            nc.gpsimd.dma_start(out=kb_sb[:], in_=kb_d[:])
            nc.gpsimd.dma_start(out=vb_sb[:], in_=vb_d[:])

            qhT = [persist.tile([P, S], mm_dt, tag=f"qhT{m}", name=f"qhT{m}") for m in range(4)]
            khT = [persist.tile([P, S], mm_dt, tag=f"khT{m}", name=f"khT{m}") for m in range(4)]
            vh = [persist.tile([P, HC, DEPTH + 1], es_dt, tag=f"vh{m}", name=f"vh{m}")
                  for m in range(8)]
            ctxT = [persist.tile([P, S], mm_dt, tag=f"ctxT{m}", name=f"ctxT{m}") for m in range(4)]
            adjT = [persist.tile([P, S], mm_dt, tag=f"adjT{m}", name=f"adjT{m}") for m in range(8)]
            dwT = [persist.tile([P, S], mm_dt, tag=f"dwT{m}", name=f"dwT{m}") for m in range(4)]

            for m in range(8):
                nc.gpsimd.dma_start(out=adjT[m][:], in_=adjT_d[m * P:(m + 1) * P, :])
            for m in range(4):
                nc.gpsimd.dma_start(out=dwT[m][:], in_=dwT_d[m * P:(m + 1) * P, :])
            for m in range(8):
                nc.vector.tensor_copy(out=vh[m][:, :, DEPTH:DEPTH + 1],
                                      in_=ones[:, 0:HC])

            # ---- Phase A: projections ----
            with tc.tile_pool(name="acts", bufs=3) as actp, \
                 tc.tile_pool(name="wts", bufs=8) as wtp, \
                 tc.tile_pool(name="psA", bufs=1, space="PSUM") as psA:

                def proj_qkT(src, w_src, bias_sb, outT):
                    ps = {}
                    for m in range(4):
                        for n in range(2):
                            ps[(m, n)] = psA.tile([P, 512], F32, tag=f"pj{m * 2 + n}", name=f"pj{m * 2 + n}")
                    for kd in range(8):
                        w_t = wtp.tile([P, W], mm_dt, tag="w")
                        nc.sync.dma_start(out=w_t[:],
                                          in_=w_src[kd * P:(kd + 1) * P, :])
                        a_t = actp.tile([P, S], mm_dt, tag="a")
                        nc.sync.dma_start(out=a_t[:],
                                          in_=src[kd * P:(kd + 1) * P, :])
                        for m in range(4):
                            for n in range(2):
                                nc.tensor.matmul(
                                    ps[(m, n)][:],
                                    w_t[:, m * P:(m + 1) * P],
                                    a_t[:, n * 512:(n + 1) * 512],
                                    start=(kd == 0), stop=(kd == 7))
                    for m in range(4):
                        for n in range(2):
                            nc.vector.tensor_scalar(
                                out=outT[m][:, n * 512:(n + 1) * 512],
                                in0=ps[(m, n)][:],
                                scalar1=bias_sb[:, m:m + 1],
                                scalar2=None, op0=ADD)

                def proj_v():
                    ps = [psA.tile([P, W], F32, tag=f"pj{i}", name=f"pj{i}") for i in range(8)]
                    for kd in range(8):
                        w_t = wtp.tile([P, W], mm_dt, tag="w")
                        nc.sync.dma_start(out=w_t[:],
                                          in_=wvT[kd * P:(kd + 1) * P, :])
                        a_t = actp.tile([P, S], mm_dt, tag="a")
                        nc.sync.dma_start(out=a_t[:],
                                          in_=vT[kd * P:(kd + 1) * P, :])
                        for m in range(8):
                            nc.tensor.matmul(
                                ps[m][:],
                                a_t[:, m * P:(m + 1) * P],
                                w_t[:],
                                start=(kd == 0), stop=(kd == 7))
                    for m in range(8):
                        nc.vector.tensor_copy(
                            out=vh[m][:, :, 0:DEPTH],
                            in_=ps[m][:].rearrange("p (h d) -> p h d", h=HC))

                proj_qkT(qT, wqT, qb_sb, qhT)
                proj_qkT(kT, wkT, kb_sb, khT)
                proj_v()

            # ---- Phase B: attention, head pairs ----
            # Heads 2j (partitions 0:64) and 2j+1 (partitions 64:128) of
            # chunk j are processed together: their K=64 logits matmuls run
            # on disjoint PE row-groups concurrently. ctx~T accumulates in
            # one-bank PSUM tiles per 4-sk chunk and is evicted to SBUF, so
            # PSUM stays shallow and the pl rings can be 3 deep -- keeping
            # the PE stream dense enough that HAM stays un-throttled.
            with tc.tile_pool(name="esb", bufs=5) as esp, \
                 tc.tile_pool(name="zsb", bufs=2) as zsp, \
                 tc.tile_pool(name="ctxa", bufs=2) as cap, \
                 tc.tile_pool(name="plp", bufs=3, space="PSUM") as plp, \
                 tc.tile_pool(name="pcp", bufs=2, space="PSUM") as pcp:
                for j in range(HC // 2):
                    hs = (2 * j, 2 * j + 1)
                    q_h = {h: qhT[j][(h % 2) * DEPTH:(h % 2 + 1) * DEPTH, :]
                           for h in hs}
                    k_h = {h: khT[j][(h % 2) * DEPTH:(h % 2 + 1) * DEPTH, :]
                           for h in hs}
                    ctxa = {h: cap.tile([DEPTH + 1, S], F32,
                                        tag=f"ctxa{h % 2}",
                                        name=f"ctxa{h % 2}") for h in hs}
                    for skc in range(2):      # sk chunks of 4
                        sks = range(4 * skc, 4 * skc + 4)
                        # pass 1: logits (PE, paired row-groups) -> +adjoin
                        # (DVE, in-place in PSUM) -> exp (ACT, PSUM->SBUF)
                        es_pair = {}
                        es_tiles = {}
                        for sk in sks:
                            pls = {}
                            for n in range(2):
                                for h in hs:
                                    pl = plp.tile([P, 512], F32,
                                                  tag=f"pl{h % 2}",
                                                  name=f"pl{h % 2}")
                                    pls[(h, n)] = pl
                                    nc.tensor.matmul(
                                        pl[:],
                                        k_h[h][:, sk * P:(sk + 1) * P],
                                        q_h[h][:, n * 512:(n + 1) * 512],
                                        start=True, stop=True)
                            for h in hs:
                                if sk % 2 == 0:
                                    es_pair[h] = esp.tile([P, 2, S], es_dt,
                                                          tag="es", name="es")
                                es_t = es_pair[h]
                                es_tiles[(h, sk)] = es_t[:, sk % 2, :]
                                for n in range(2):
                                    pl = pls[(h, n)]
                                    nc.vector.tensor_tensor(
                                        out=pl[:], in0=pl[:],
                                        in1=adjT[sk][:, n * 512:(n + 1) * 512],
                                        op=ADD)
                                    nc.scalar.activation(
                                        es_t[:, sk % 2, n * 512:(n + 1) * 512],
                                        pl[:], EXP)
                                if sk % 2 == 1:
                                    a0 = (sk - 1) * P
                                    dst = attn_un[h, a0:a0 + 2 * P, :]
                                    dst = dst.rearrange("(s p) e -> p s e",
                                                        p=P)
                                    nc.sync.dma_start(out=dst, in_=es_t[:])
                        # pass 2: ctx~T accumulation into one-bank PSUM
                        # tiles, evicted to SBUF per chunk
                        for h in hs:
                            for n in range(2):
                                pc = pcp.tile([DEPTH + 1, 512], F32,
                                              tag="pc", name="pc")
                                for sk in sks:
                                    nc.tensor.matmul(
                                        pc[:],
                                        vh[sk][:, h, :],
                                        es_tiles[(h, sk)][:, n * 512:(n + 1) * 512],
                                        start=(sk % 4 == 0), stop=(sk % 4 == 3))
                                dstc = ctxa[h][:, n * 512:(n + 1) * 512]
                                if skc == 0:
                                    nc.vector.tensor_copy(out=dstc, in_=pc[:])
                                else:
                                    nc.vector.tensor_tensor(
                                        out=dstc, in0=dstc, in1=pc[:], op=ADD)
                    # Z -> 1/Z -> partition broadcast -> normalize ctx~T
                    for h in hs:
                        poff = (h % 2) * DEPTH
                        ca = ctxa[h]
                        zz = zsp.tile([1, S], F32, tag="zz")
                        nc.vector.tensor_copy(out=zz[0:1, :],
                                              in_=ca[DEPTH:DEPTH + 1, :])
                        rz = zsp.tile([1, S], F32, tag="rz")
                        nc.vector.reciprocal_approx_fast(out=rz[0:1, :],
                                                         in_=zz[0:1, :])
                        rzb = zsp.tile([DEPTH, S], F32, tag="rzb")
                        nc.gpsimd.partition_broadcast(rzb[:], rz[0:1, :],
                                                      channels=DEPTH)
                        ct = ctxT[j]
                        nc.vector.tensor_tensor(out=ct[poff:poff + DEPTH, :],
                                                in0=ca[0:DEPTH, :],
                                                in1=rzb[:], op=MULT)
                        nc.vector.tensor_scalar(
                            out=ct[poff:poff + DEPTH, :],
                            in0=ct[poff:poff + DEPTH, :],
                            scalar1=vb_sb[poff:poff + DEPTH, j:j + 1],
                            scalar2=None, op0=ADD)

            # ---- Phase C: dense (row-parallel partial) ----
            with tc.tile_pool(name="osb", bufs=3) as osp, \
                 tc.tile_pool(name="psC", bufs=4, space="PSUM") as psC:
                for m in range(8):
                    os_t = osp.tile([P, S], F32, tag="os")
                    for n in range(2):
                        pd = psC.tile([P, 512], F32, tag="pd")
                        for kc in range(4):
                            nc.tensor.matmul(
                                pd[:],
                                ctxT[kc][:, m * P:(m + 1) * P],
                                dwT[kc][:, n * 512:(n + 1) * 512],
                                start=(kc == 0), stop=(kc == 3))
                        nc.vector.tensor_copy(
                            out=os_t[:, n * 512:(n + 1) * 512], in_=pd[:])
                    nc.sync.dma_start(out=outp[m * P:(m + 1) * P, :],
                                      in_=os_t[:])

    nc.compile()
    return nc


_PROGRAM_CACHE = {}


def get_program(mm_dt=MM_DT, es_dt=None):
    key = (str(mm_dt), str(es_dt))
    if key not in _PROGRAM_CACHE:
        _PROGRAM_CACHE[key] = build_program(mm_dt, es_dt)
    return _PROGRAM_CACHE[key]


def make_in_maps(v, k, q, mask, adjoin_matrix,
                 wq_w, wq_b, wk_w, wk_b, wv_w, wv_b, dense_w, dense_b):
    c = np.ascontiguousarray
    f32 = np.float32
    in_maps = []
    per_batch = {}
    for b in range(4):
        per_batch[b] = {
            "qT": c(np.asarray(q[b], f32).T),
            "kT": c(np.asarray(k[b], f32).T),
            "vT": c(np.asarray(v[b], f32).T),
            "adjT": c(np.asarray(adjoin_matrix[b, 0], f32).T)
            + np.float32(-1e9) * np.asarray(mask[b, 0, 0], f32)[:, None],
        }
    for cid in range(8):
        b, g = cid // 2, cid % 2
        gs = slice(g * W, (g + 1) * W)
        m = dict(per_batch[b])
        m["wqT"] = c(np.asarray(wq_w, f32)[gs].T) * f32(0.125)
        m["wkT"] = c(np.asarray(wk_w, f32)[gs].T)
        m["wvT"] = c(np.asarray(wv_w, f32)[gs].T)
        m["dwT"] = c(np.asarray(dense_w, f32)[:, gs].T)
        m["qb"] = c((np.asarray(wq_b, f32)[gs] * f32(0.125)).reshape(4, P).T)
        m["kb"] = c(np.asarray(wk_b, f32)[gs].reshape(4, P).T)
        m["vb"] = c(np.asarray(wv_b, f32)[gs].reshape(4, P).T)
        in_maps.append(m)
    return in_maps


def assemble_outputs(results, dense_b):
    out = np.empty((4, S, D), np.float32)
    attn = np.empty((4, H, S, S), np.float32)
    for cid in range(8):
        b, g = cid // 2, cid % 2
        au = results[cid]["attn_un"]          # [HC, sk, sq]
        z = au.sum(axis=1)                    # [HC, sq]
        attn[b, g * HC:(g + 1) * HC] = (au / z[:, None, :]).transpose(0, 2, 1)
    db = np.asarray(dense_b, np.float32)
    for b in range(4):
        out[b] = results[2 * b]["outp"] + results[2 * b + 1]["outp"] + db
    return out, attn


def run_cores(inputs, mm_dt=MM_DT, es_dt=None, trace=False, **run_kwargs):
    nc = get_program(mm_dt, es_dt)
    in_maps = make_in_maps(**inputs)
    res = run_bass_kernel_spmd(nc, in_maps, core_ids=list(range(8)),
                               trace=trace, **run_kwargs)
    return res


def kernel(**inputs):
    res = run_cores(inputs)
    return assemble_outputs(res.results, inputs["dense_b"])
